# revision 38
# baseline (speedup 1.0000x reference)
"""Multi-head self-attention (B=4, T=2048, D=1024, H=16) on 8 Trainium2 cores.

Sharding: core c = 2*b + s owns batch b (of 4) and head-half s (heads
8s..8s+7).  Each core computes QKV + attention for its 8 heads in a
transposed layout, 2-rank AllGathers within each batch-pair re-shard the
attention output from head-split to token-split, and the final projection
produces a disjoint [1024-token, 1024] slice of the output per core.

Attention layout per head pair (packed on SBUF partitions 0-63 / 64-127):
  S_T[tk, tq] = K_T.T @ Q_T   (two heads row-packed on the PE array)
  P = exp(S_T / 8)            (ScalarE, scale folded into the activation)
  [O_T; denom] = [V | 1].T @ P_T   (ones column yields softmax denominators)

The attention inner loop is ScalarE(exp)-paced (~1.1us per key tile); QKV
projection matmuls for group g+1 (and for the last group, the main pass of
the output projection) are drip-fed one op per key tile into the exp-paced
bubbles on the PE.

kernel(x, w_qkv, w_proj) -> [4, 2048, 1024] float32
"""

import sys

sys.path.insert(0, "/opt/trn_rl_repo")

import numpy as np
import ml_dtypes

import concourse.bass as bass
import concourse.bacc as bacc
import concourse.mybir as mybir
import concourse.tile as tile
from concourse.bass_utils import run_bass_kernel_spmd
from concourse.masks import make_identity

BF16 = mybir.dt.bfloat16
F32 = mybir.dt.float32
F16 = mybir.dt.float16

P = 128      # partitions
T = 2048     # sequence length
TH = T // 2  # token half
D = 1024     # model dim
DH = 64      # head dim
NG = 4       # head pair-groups per core (2 heads each = 8 heads)
NCH = 4      # tq chunks of 512 per sequence
CH = 512     # tq chunk size
NTK = T // P  # 16 key tiles
ND = D // P   # 8 d-tiles
NE = D // P   # 8 e-tiles
N_CORES = 8

_CACHE = {}


def build_kernel(num_devices=N_CORES, use_a2a=True):
    nc = bacc.Bacc(num_devices=num_devices)

    xt = nc.dram_tensor("xt", [D, T], BF16, kind="ExternalInput")
    wq = nc.dram_tensor("wq", [D, NG * P], BF16, kind="ExternalInput")
    wk = nc.dram_tensor("wk", [D, NG * P], BF16, kind="ExternalInput")
    wv = nc.dram_tensor("wv", [D, NG * P], BF16, kind="ExternalInput")
    wp = nc.dram_tensor("wp", [D if use_a2a else D // 2, D], BF16,
                        kind="ExternalInput")
    if use_a2a:
        y = nc.dram_tensor("y", [D, TH], F32, kind="ExternalOutput")
    else:
        y = nc.dram_tensor("y", [D, T], F16, kind="ExternalOutput")

    with tile.TileContext(nc) as tc:
        with (
            tc.tile_pool(name="const", bufs=1) as cpool,
            tc.tile_pool(name="wpool", bufs=1) as wpool,
            tc.tile_pool(name="xpool", bufs=1) as xpool,
            tc.tile_pool(name="qkpool", bufs=2) as qkpool,
            tc.tile_pool(name="vpool", bufs=2) as vpool,
            tc.tile_pool(name="ptpool", bufs=6) as ptpool,
            tc.tile_pool(name="otpool", bufs=1) as otpool,
            tc.tile_pool(name="opool", bufs=9) as opool,
            tc.tile_pool(name="rpool", bufs=4) as rpool,
            tc.tile_pool(name="collpool", bufs=2) as collpool,
            tc.tile_pool(name="ypool", bufs=6) as ypool,
            tc.tile_pool(name="ypartpool", bufs=16 if use_a2a else 32) as ypartpool,
            tc.tile_pool(name="projpool", bufs=1) as projpool,
            tc.tile_pool(name="ps_s", bufs=2, space="PSUM") as ps_s,
            tc.tile_pool(name="ps_pv", bufs=2, space="PSUM") as ps_pv,
            tc.tile_pool(name="ps_acc", bufs=2, space="PSUM") as ps_acc,
            tc.tile_pool(name="dram", bufs=1, space="DRAM") as dpool,
        ):
            wk_sb = wpool.tile([P, ND, NG * P], BF16, tag="wk")
            nc.sync.dma_start(wk_sb, wk.rearrange("(a p) b -> p a b", p=P))
            x_sb = []
            for d in range(ND):
                xd = xpool.tile([P, T], BF16, tag=f"x{d}")
                nc.sync.dma_start(xd, xt[d * P:(d + 1) * P, :])
                x_sb.append(xd)
            wq_sb = wpool.tile([P, ND, NG * P], BF16, tag="wq")
            wv_sb = wpool.tile([P, ND, NG * P], BF16, tag="wv")
            nc.sync.dma_start(wq_sb, wq.rearrange("(a p) b -> p a b", p=P))
            nc.sync.dma_start(wv_sb, wv.rearrange("(a p) b -> p a b", p=P))

            ident = cpool.tile([P, P], BF16, tag="ident")
            make_identity(nc, ident)

            if use_a2a:
                wp_sb = projpool.tile([P, ND, D], BF16, tag="wp")
                nc.sync.dma_start(wp_sb, wp.rearrange("(a p) e -> p a e", p=P))
                at_sb = projpool.tile([P, ND, TH], BF16, tag="at")
            else:
                wp_sb = projpool.tile([P, NG, D], BF16, tag="wp")
                nc.sync.dma_start(wp_sb, wp.rearrange("(a p) e -> p a e", p=P))

            # attention output, transposed: [dh-pair(128), g*2048 + tq]
            ot_sb = otpool.tile([P, NG * T], BF16, tag="ot")

            qkv_tiles = {}  # g -> (qt, kt, vt, v_sb)
            last_qk = [None]

            def alloc_group(g):
                qkv_tiles[g] = (
                    qkpool.tile([P, T], BF16, tag="qt", name=f"qt{g}"),
                    qkpool.tile([P, T], BF16, tag="kt", name=f"kt{g}"),
                    qkpool.tile([P, T], BF16, tag="vt", name=f"vt{g}"),
                    vpool.tile([P, NTK * 130], BF16, tag="v", name=f"v{g}"),
                )
                nc.gpsimd.memset(qkv_tiles[g][3], 1.0)

            def qkv_gen(g):
                """One-PE-op-at-a-time generator: QKV projections (d-outer,
                chunk-paired so the first group pipelines with x DMAs),
                then the V transpose to natural layout."""
                gc = slice(g * P, (g + 1) * P)
                for which in (1, 0, 2):
                    wsb = (wq_sb, wk_sb, wv_sb)[which]
                    dst = qkv_tiles[g][which]
                    for c in range(NCH):
                        psq = ps_acc.tile([P, CH], F32, tag="acc",
                                          name=f"qkv{g}_{which}_{c}")
                        for d in range(ND):
                            nc.tensor.matmul(
                                psq, lhsT=wsb[:, d, gc],
                                rhs=x_sb[d][:, c * CH:(c + 1) * CH],
                                start=(d == 0), stop=(d == ND - 1),
                            )
                            yield
                        nc.vector.tensor_copy(
                            dst[:, c * CH:(c + 1) * CH], psq
                        )
                        yield
                vt, v_sb = qkv_tiles[g][2], qkv_tiles[g][3]
                for tk in range(NTK):
                    pst = ps_acc.tile([P, P], BF16, tag="acc",
                                      name=f"vtr{g}_{tk}")
                    nc.tensor.transpose(pst, vt[:, tk * P:(tk + 1) * P], ident)
                    yield
                    nc.vector.tensor_copy(
                        v_sb[:, tk * 130:tk * 130 + 64], pst[:, 0:64]
                    )
                    nc.vector.tensor_copy(
                        v_sb[:, tk * 130 + 65:tk * 130 + 129], pst[:, 64:128]
                    )
                    yield

            y_parts = {}

            def proj_a_gen():
                """Main partial-projection pass over this core's head groups
                0-2 (normalized early), drip-fed into group 3's bubbles."""
                if use_a2a:
                    units = [(e, chh, (0, 4, 1, 5),
                              at_sb[:, 0, 0:1])  # placeholder
                             for e in range(NE) for chh in range(2)]
                order = ([(e, ch) for hh in range(2) for e in range(NE)
                          for ch in (2 * hh, 2 * hh + 1)]
                         if not use_a2a else
                         [(e, ch) for e in range(NE) for ch in range(2)])
                for e, ch in order:
                    if True:
                        ec = slice(e * P, (e + 1) * P)
                        psy = ps_acc.tile([P, CH], F32, tag="acc",
                                          name=f"pa{e}_{ch}")
                        if use_a2a:
                            dseq = (0, 4, 1, 5)
                            for k, dd in enumerate(dseq):
                                nc.tensor.matmul(
                                    psy, lhsT=wp_sb[:, dd, ec],
                                    rhs=at_sb[:, dd, ch * CH:(ch + 1) * CH],
                                    start=(k == 0), stop=(k == len(dseq) - 1),
                                )
                                yield
                        else:
                            for k in range(3):
                                nc.tensor.matmul(
                                    psy, lhsT=wp_sb[:, k, ec],
                                    rhs=ot_sb[:, k * T + ch * CH:
                                              k * T + (ch + 1) * CH],
                                    start=(k == 0), stop=(k == 2),
                                )
                                yield
                        part = ypartpool.tile([P, CH], F16, tag="ypart",
                                              name=f"yp{e}_{ch}")
                        nc.vector.tensor_copy(part, psy)
                        y_parts[(e, ch)] = part
                        yield

            def attn_chunk(g, ch, coll, ost, feed):
                qt, kt, _, v_sb = qkv_tiles[g]
                qs0 = qt[0:64, ch * CH:(ch + 1) * CH]
                qs1 = qt[64:128, ch * CH:(ch + 1) * CH]
                pv0 = ps_pv.tile([P, CH], F32, tag="pv", name=f"pv0_{g}_{ch}")
                pv1 = ps_pv.tile([P, CH], F32, tag="pv", name=f"pv1_{g}_{ch}")
                for tk in range(NTK):
                    feed(ch * NTK + tk)
                    ps = ps_s.tile([P, 2 * CH], F32, tag="s",
                                   name=f"s{g}_{ch}_{tk}")
                    qk0 = nc.tensor.matmul(
                        ps[:, 0:CH],
                        lhsT=kt[0:64, tk * P:(tk + 1) * P],
                        rhs=qs0, start=True, stop=True,
                    )
                    qk1 = nc.tensor.matmul(
                        ps[:, CH:2 * CH],
                        lhsT=kt[64:128, tk * P:(tk + 1) * P],
                        rhs=qs1, start=True, stop=True,
                    )
                    last_qk[0] = qk1.ins
                    pt = ptpool.tile([P, 2 * CH], BF16, tag="pt")
                    nc.scalar.activation(
                        pt, ps, mybir.ActivationFunctionType.Exp, scale=0.125
                    )
                    nc.tensor.matmul(
                        pv0[0:65, :],
                        lhsT=v_sb[:, tk * 130:tk * 130 + 65],
                        rhs=pt[:, 0:CH],
                        start=(tk == 0), stop=(tk == NTK - 1),
                    )
                    nc.tensor.matmul(
                        pv1[0:65, :],
                        lhsT=v_sb[:, tk * 130 + 65:tk * 130 + 130],
                        rhs=pt[:, CH:2 * CH],
                        start=(tk == 0), stop=(tk == NTK - 1),
                    )
                # free the pv psums quickly (high priority so these DVE
                # copies jump ahead of filler casts and the normalize chain).
                # Denominator rows land 32-aligned in the half's collector.
                for h, pv in ((0, pv0), (1, pv1)):
                    r = 2 * (ch % 2) + h
                    with tc.high_priority():
                        nc.vector.tensor_copy(
                            coll[ch // 2][r * 32:r * 32 + 1, :], pv[64:65, :]
                        )
                        o = opool.tile([64, CH], F32, tag="ost")
                        nc.vector.tensor_copy(o, pv[0:64, :])
                    ost[2 * ch + h] = o

            def emit_passB(chs):
                # pass B: group 3's contribution + combine + store (fp16
                # partials; the host sums the two partials of each pair)
                for e in range(NE):
                    ec = slice(e * P, (e + 1) * P)
                    for ch in chs:
                        psy = ps_acc.tile([P, CH], F32, tag="acc",
                                          name=f"pb{e}_{ch}")
                        nc.tensor.matmul(
                            psy, lhsT=wp_sb[:, 3, ec],
                            rhs=ot_sb[:, 3 * T + ch * CH:3 * T + (ch + 1) * CH],
                            start=True, stop=True,
                        )
                        ysb = ypool.tile([P, CH], F16, tag="ysb")
                        nc.vector.tensor_add(ysb, y_parts[(e, ch)], psy)
                        nc.sync.dma_start(
                            y[e * P:(e + 1) * P, ch * CH:(ch + 1) * CH], ysb
                        )

            groups = [list(range(num_devices))]
            ag_outs = {}

            def half_finalize(g, hf, coll, ost):
                """Normalize token-half hf of group g (batched reciprocal)
                and kick its AllGather."""
                nc.vector.reciprocal(coll[hf], coll[hf])
                for ch in (2 * hf, 2 * hf + 1):
                    for h in range(2):
                        r = 2 * (ch % 2) + h
                        rs = rpool.tile([1, CH], F32, tag="rs")
                        nc.vector.tensor_copy(
                            rs, coll[hf][r * 32:r * 32 + 1, :]
                        )
                        rb = rpool.tile([64, CH], F32, tag="rb")
                        nc.gpsimd.partition_broadcast(rb, rs)
                        cols = g * T + ch * CH
                        nc.vector.tensor_mul(
                            ot_sb[h * 64:(h + 1) * 64, cols:cols + CH],
                            ost[2 * ch + h], rb,
                        )
                if not use_a2a:
                    return
                ag_in = dpool.tile([P, TH], BF16, tag=f"ag_in{g}_{hf}")
                nc.sync.dma_start(
                    ag_in, ot_sb[:, g * T + hf * TH:g * T + (hf + 1) * TH]
                )
                nv = num_devices
                nc.gpsimd.collective_compute(
                    "AllGather",
                    mybir.AluOpType.bypass,
                    replica_groups=groups,
                    ins=[ag_in.opt()],
                    outs=[ag_outs[g][hf * nv:(hf + 1) * nv].opt()],
                )
                if hf == 1:
                    # both halves gathered: load the at_sb d-tiles this core
                    # needs: rows (own_half*nv + pair_base + j) of the
                    # [2*nv, P, TH] gather buffer, dynamic by rank.
                    for j in range(2):
                        nc.sync.dma_start(
                            at_sb[:, 4 * j + g, :],
                            ag_outs[g][bass.ds(row_base + j, 1)],
                        )

            if use_a2a:
                pid_sv = nc.sync.partition_id()
                own_half = pid_sv % 2
                # row own_half*nv + (pid - own_half) + j = pid + own_half*(nv-1) + j
                row_base = pid_sv + own_half * (num_devices - 1)
                for g in range(NG):
                    # [token-half x rank, P, TH]
                    ag_outs[g] = dpool.tile(
                        [2 * num_devices, P, TH], BF16,
                        tag=f"ag_out{g}", name=f"ago{g}"
                    )

            # ---- emission ----
            alloc_group(0)
            for _ in qkv_gen(0):
                pass

            for g in range(NG):
                if g + 1 < NG:
                    alloc_group(g + 1)
                    gen, n_ops = qkv_gen(g + 1), 3 * NCH * (ND + 1) + NTK * 2
                elif use_a2a:
                    gen, n_ops = proj_a_gen(), NE * 2 * 5
                else:
                    gen, n_ops = proj_a_gen(), NE * NCH * 4
                emitted = [0]

                # no fillers in the first two key-tiles of each chunk
                # (pv-alloc / epilogue transitions congest the PE there)
                fw = [0 if (t % NTK) < 2 else 1 for t in range(NCH * NTK)]
                fcum = [0]
                for w in fw:
                    fcum.append(fcum[-1] + w)

                def feed(i, gen=gen, n_ops=n_ops, emitted=emitted, fcum=fcum):
                    target = n_ops * fcum[i + 1] // fcum[-1]
                    while emitted[0] < target:
                        next(gen, None)
                        emitted[0] += 1

                coll = [
                    collpool.tile([97, CH], F32, tag="coll0", name=f"coll0_{g}"),
                    collpool.tile([97, CH], F32, tag="coll1", name=f"coll1_{g}"),
                ]
                nc.gpsimd.memset(coll[0], 1.0)
                nc.gpsimd.memset(coll[1], 1.0)
                ost = {}
                for ch in range(NCH):
                    attn_chunk(g, ch, coll, ost, feed)
                    if not use_a2a and g == NG - 1 and ch == 2:
                        emit_passB((0, 1))
                    if ch == 1:
                        half_finalize(g, 0, coll, ost)
                    elif ch == 3:
                        half_finalize(g, 1, coll, ost)
                for _ in gen:
                    pass

            if use_a2a:
                # projection pass B: last group's d-tiles + combine + store
                for e in range(NE):
                    ec = slice(e * P, (e + 1) * P)
                    for chh in range(2):
                        psy = ps_acc.tile([P, CH], F32, tag="acc",
                                          name=f"pb{e}_{chh}")
                        dseq = (2, 6, 3, 7)
                        for k, dd in enumerate(dseq):
                            nc.tensor.matmul(
                                psy, lhsT=wp_sb[:, dd, ec],
                                rhs=at_sb[:, dd, chh * CH:(chh + 1) * CH],
                                start=(k == 0), stop=(k == len(dseq) - 1),
                            )
                        ysb = ypool.tile([P, CH], F32, tag="ysb")
                        nc.vector.tensor_add(ysb, y_parts[(e, chh)], psy)
                        nc.sync.dma_start(
                            y[e * P:(e + 1) * P, chh * CH:(chh + 1) * CH], ysb
                        )
            else:
                emit_passB((2, 3))

    nc.compile()
    return nc


def shard_inputs(x, w_qkv, w_proj, use_a2a=True):
    """Build the 8 per-core in_maps (host-side sharding + transposes)."""
    bf16 = ml_dtypes.bfloat16
    wp_t = np.ascontiguousarray(w_proj.T).astype(bf16)  # [d, e]
    in_maps = []
    for c in range(N_CORES):
        b, s = divmod(c, 2)
        xt = np.ascontiguousarray(x[b].T).astype(bf16)  # [D, T]
        heads = [8 * s + 2 * g for g in range(NG)]

        def wslice(base):
            cols = [
                w_qkv[base + h * DH: base + (h + 2) * DH, :] for h in heads
            ]
            return np.ascontiguousarray(np.concatenate(cols, axis=0).T).astype(bf16)

        m = {
            "xt": xt,
            "wq": wslice(0),
            "wk": wslice(D),
            "wv": wslice(2 * D),
        }
        if use_a2a:
            m["wp"] = wp_t
        else:
            rows = np.concatenate(
                [w_proj[:, (8 * s + 2 * g) * DH:(8 * s + 2 * g + 2) * DH].T
                 for g in range(NG)], axis=0
            )
            m["wp"] = np.ascontiguousarray(rows).astype(bf16)
        in_maps.append(m)
    return in_maps


def assemble_output(results, use_a2a=True):
    out = np.empty((4, T, D), dtype=np.float32)
    if use_a2a:
        for c in range(N_CORES):
            b, s = divmod(c, 2)
            out[b, s * TH:(s + 1) * TH, :] = results[c]["y"].T
    else:
        for b in range(4):
            acc = (results[2 * b]["y"].astype(np.float32)
                   + results[2 * b + 1]["y"].astype(np.float32))
            out[b] = acc.T
    return out


def run(x, w_qkv, w_proj, use_a2a=False, trace=False):
    key = ("k", use_a2a)
    if key not in _CACHE:
        _CACHE[key] = build_kernel(use_a2a=use_a2a)
    nc = _CACHE[key]
    in_maps = shard_inputs(x, w_qkv, w_proj, use_a2a=use_a2a)
    res = run_bass_kernel_spmd(
        nc, in_maps, core_ids=list(range(N_CORES)), trace=trace
    )
    return assemble_output(res.results, use_a2a=use_a2a), res


def kernel(x, w_qkv, w_proj):
    x = np.asarray(x, dtype=np.float32)
    w_qkv = np.asarray(w_qkv, dtype=np.float32)
    w_proj = np.asarray(w_proj, dtype=np.float32)
    out, _ = run(x, w_qkv, w_proj)
    return out


# revision 39
# speedup vs baseline: 1.1855x; 1.1855x over previous
"""Multi-head self-attention (B=4, T=2048, D=1024, H=16) on 8 Trainium2 cores.

Sharding: core c = 2*b + s owns batch b (of 4) and head-half s (heads
8s..8s+7).  Each core computes QKV + attention for its 8 heads in a
transposed layout, 2-rank AllGathers within each batch-pair re-shard the
attention output from head-split to token-split, and the final projection
produces a disjoint [1024-token, 1024] slice of the output per core.

Attention layout per head pair (packed on SBUF partitions 0-63 / 64-127):
  S_T[tk, tq] = K_T.T @ Q_T   (two heads row-packed on the PE array)
  P = exp(S_T / 8)            (ScalarE, scale folded into the activation)
  [O_T; denom] = [V | 1].T @ P_T   (ones column yields softmax denominators)

The attention inner loop is ScalarE(exp)-paced (~1.1us per key tile); QKV
projection matmuls for group g+1 (and for the last group, the main pass of
the output projection) are drip-fed one op per key tile into the exp-paced
bubbles on the PE.

kernel(x, w_qkv, w_proj) -> [4, 2048, 1024] float32
"""

import sys

sys.path.insert(0, "/opt/trn_rl_repo")

import numpy as np
import ml_dtypes

import concourse.bass as bass
import concourse.bacc as bacc
import concourse.mybir as mybir
import concourse.tile as tile
from concourse.bass_utils import run_bass_kernel_spmd
from concourse.masks import make_identity

BF16 = mybir.dt.bfloat16
F32 = mybir.dt.float32
F16 = mybir.dt.float16

P = 128      # partitions
T = 2048     # sequence length
TH = T // 2  # token half
D = 1024     # model dim
DH = 64      # head dim
NG = 4       # head pair-groups per core (2 heads each = 8 heads)
NCH = 4      # tq chunks of 512 per sequence
CH = 512     # tq chunk size
NTK = T // P  # 16 key tiles
ND = D // P   # 8 d-tiles
NE = D // P   # 8 e-tiles
N_CORES = 8

_CACHE = {}


def build_kernel(num_devices=N_CORES, use_a2a=True):
    nc = bacc.Bacc(num_devices=num_devices)

    xt = nc.dram_tensor("xt", [D, T], BF16, kind="ExternalInput")
    wq = nc.dram_tensor("wq", [D, NG * P], BF16, kind="ExternalInput")
    wk = nc.dram_tensor("wk", [D, NG * P], BF16, kind="ExternalInput")
    wv = nc.dram_tensor("wv", [D, NG * P], BF16, kind="ExternalInput")
    wp = nc.dram_tensor("wp", [D if use_a2a else D // 2, D], BF16,
                        kind="ExternalInput")
    if use_a2a:
        y = nc.dram_tensor("y", [D, TH], F32, kind="ExternalOutput")
    else:
        y = nc.dram_tensor("y", [D, T], F16, kind="ExternalOutput")

    with tile.TileContext(nc) as tc:
        with (
            tc.tile_pool(name="const", bufs=1) as cpool,
            tc.tile_pool(name="wpool", bufs=1) as wpool,
            tc.tile_pool(name="xpool", bufs=1) as xpool,
            tc.tile_pool(name="qkpool", bufs=2) as qkpool,
            tc.tile_pool(name="vpool", bufs=2) as vpool,
            tc.tile_pool(name="ptpool", bufs=6) as ptpool,
            tc.tile_pool(name="otpool", bufs=1) as otpool,
            tc.tile_pool(name="opool", bufs=9) as opool,
            tc.tile_pool(name="rpool", bufs=4) as rpool,
            tc.tile_pool(name="collpool", bufs=2) as collpool,
            tc.tile_pool(name="ypool", bufs=6) as ypool,
            tc.tile_pool(name="ypartpool", bufs=16 if use_a2a else 32) as ypartpool,
            tc.tile_pool(name="projpool", bufs=1) as projpool,
            tc.tile_pool(name="ps_s", bufs=2, space="PSUM") as ps_s,
            tc.tile_pool(name="ps_pv", bufs=2, space="PSUM") as ps_pv,
            tc.tile_pool(name="ps_acc", bufs=2, space="PSUM") as ps_acc,
            tc.tile_pool(name="dram", bufs=1, space="DRAM") as dpool,
        ):
            wk_sb = wpool.tile([P, ND, NG * P], BF16, tag="wk")
            nc.sync.dma_start(wk_sb, wk.rearrange("(a p) b -> p a b", p=P))
            x_sb = []
            for d in range(ND):
                xd = xpool.tile([P, T], BF16, tag=f"x{d}")
                nc.sync.dma_start(xd, xt[d * P:(d + 1) * P, :])
                x_sb.append(xd)
            wq_sb = wpool.tile([P, ND, NG * P], BF16, tag="wq")
            wv_sb = wpool.tile([P, ND, NG * P], BF16, tag="wv")
            nc.sync.dma_start(wq_sb, wq.rearrange("(a p) b -> p a b", p=P))
            nc.sync.dma_start(wv_sb, wv.rearrange("(a p) b -> p a b", p=P))

            ident = cpool.tile([P, P], BF16, tag="ident")
            make_identity(nc, ident)

            if use_a2a:
                wp_sb = projpool.tile([P, ND, D], BF16, tag="wp")
                nc.sync.dma_start(wp_sb, wp.rearrange("(a p) e -> p a e", p=P))
                at_sb = projpool.tile([P, ND, TH], BF16, tag="at")
            else:
                wp_sb = projpool.tile([P, NG, D], BF16, tag="wp")
                nc.sync.dma_start(wp_sb, wp.rearrange("(a p) e -> p a e", p=P))

            # attention output, transposed: [dh-pair(128), g*2048 + tq]
            ot_sb = otpool.tile([P, NG * T], BF16, tag="ot")

            qkv_tiles = {}  # g -> (qt, kt, vt, v_sb)
            last_qk = [None]

            def alloc_group(g):
                qkv_tiles[g] = (
                    qkpool.tile([P, T], BF16, tag="qt", name=f"qt{g}"),
                    qkpool.tile([P, T], BF16, tag="kt", name=f"kt{g}"),
                    qkpool.tile([P, T], BF16, tag="vt", name=f"vt{g}"),
                    vpool.tile([P, NTK * 130], BF16, tag="v", name=f"v{g}"),
                )
                nc.gpsimd.memset(qkv_tiles[g][3], 1.0)

            def qkv_gen(g):
                """One-PE-op-at-a-time generator: QKV projections (d-outer,
                chunk-paired so the first group pipelines with x DMAs),
                then the V transpose to natural layout."""
                gc = slice(g * P, (g + 1) * P)
                for which in (1, 0, 2):
                    wsb = (wq_sb, wk_sb, wv_sb)[which]
                    dst = qkv_tiles[g][which]
                    for c in range(NCH):
                        psq = ps_acc.tile([P, CH], F32, tag="acc",
                                          name=f"qkv{g}_{which}_{c}")
                        for d in range(ND):
                            nc.tensor.matmul(
                                psq, lhsT=wsb[:, d, gc],
                                rhs=x_sb[d][:, c * CH:(c + 1) * CH],
                                start=(d == 0), stop=(d == ND - 1),
                            )
                            yield
                        nc.vector.tensor_copy(
                            dst[:, c * CH:(c + 1) * CH], psq
                        )
                        yield
                vt, v_sb = qkv_tiles[g][2], qkv_tiles[g][3]
                for tk in range(NTK):
                    pst = ps_acc.tile([P, P], BF16, tag="acc",
                                      name=f"vtr{g}_{tk}")
                    nc.tensor.transpose(pst, vt[:, tk * P:(tk + 1) * P], ident)
                    yield
                    nc.vector.tensor_copy(
                        v_sb[:, tk * 130:tk * 130 + 64], pst[:, 0:64]
                    )
                    nc.vector.tensor_copy(
                        v_sb[:, tk * 130 + 65:tk * 130 + 129], pst[:, 64:128]
                    )
                    yield

            y_parts = {}

            def proj_a_gen():
                """Main partial-projection pass over this core's head groups
                0-2 (normalized early), drip-fed into group 3's bubbles."""
                if use_a2a:
                    units = [(e, chh, (0, 4, 1, 5),
                              at_sb[:, 0, 0:1])  # placeholder
                             for e in range(NE) for chh in range(2)]
                order = ([(e, ch) for hh in range(2) for e in range(NE)
                          for ch in (2 * hh, 2 * hh + 1)]
                         if not use_a2a else
                         [(e, ch) for e in range(NE) for ch in range(2)])
                for e, ch in order:
                    if True:
                        ec = slice(e * P, (e + 1) * P)
                        psy = ps_acc.tile([P, CH], F32, tag="acc",
                                          name=f"pa{e}_{ch}")
                        if use_a2a:
                            dseq = (0, 4, 1, 5)
                            for k, dd in enumerate(dseq):
                                nc.tensor.matmul(
                                    psy, lhsT=wp_sb[:, dd, ec],
                                    rhs=at_sb[:, dd, ch * CH:(ch + 1) * CH],
                                    start=(k == 0), stop=(k == len(dseq) - 1),
                                )
                                yield
                        else:
                            for k in range(3):
                                nc.tensor.matmul(
                                    psy, lhsT=wp_sb[:, k, ec],
                                    rhs=ot_sb[:, k * T + ch * CH:
                                              k * T + (ch + 1) * CH],
                                    start=(k == 0), stop=(k == 2),
                                )
                                yield
                        part = ypartpool.tile([P, CH], F16, tag="ypart",
                                              name=f"yp{e}_{ch}")
                        nc.vector.tensor_copy(part, psy)
                        y_parts[(e, ch)] = part
                        yield

            def attn_chunk(g, ch, coll, ost, feed):
                qt, kt, _, v_sb = qkv_tiles[g]
                qs0 = qt[0:64, ch * CH:(ch + 1) * CH]
                qs1 = qt[64:128, ch * CH:(ch + 1) * CH]
                pv0 = ps_pv.tile([P, CH], F32, tag="pv", name=f"pv0_{g}_{ch}")
                pv1 = ps_pv.tile([P, CH], F32, tag="pv", name=f"pv1_{g}_{ch}")
                for tk in range(NTK):
                    feed(ch * NTK + tk)
                    ps = ps_s.tile([P, 2 * CH], F32, tag="s",
                                   name=f"s{g}_{ch}_{tk}")
                    qk0 = nc.tensor.matmul(
                        ps[:, 0:CH],
                        lhsT=kt[0:64, tk * P:(tk + 1) * P],
                        rhs=qs0, start=True, stop=True,
                    )
                    qk1 = nc.tensor.matmul(
                        ps[:, CH:2 * CH],
                        lhsT=kt[64:128, tk * P:(tk + 1) * P],
                        rhs=qs1, start=True, stop=True,
                    )
                    last_qk[0] = qk1.ins
                    pt = ptpool.tile([P, 2 * CH], BF16, tag="pt")
                    nc.scalar.activation(
                        pt, ps, mybir.ActivationFunctionType.Exp, scale=0.125
                    )
                    nc.tensor.matmul(
                        pv0[0:65, :],
                        lhsT=v_sb[:, tk * 130:tk * 130 + 65],
                        rhs=pt[:, 0:CH],
                        start=(tk == 0), stop=(tk == NTK - 1),
                    )
                    nc.tensor.matmul(
                        pv1[0:65, :],
                        lhsT=v_sb[:, tk * 130 + 65:tk * 130 + 130],
                        rhs=pt[:, CH:2 * CH],
                        start=(tk == 0), stop=(tk == NTK - 1),
                    )
                # free the pv psums quickly (high priority so these DVE
                # copies jump ahead of filler casts and the normalize chain).
                # Denominator rows land 32-aligned in the half's collector.
                for h, pv in ((0, pv0), (1, pv1)):
                    r = 2 * (ch % 2) + h
                    with tc.high_priority():
                        nc.vector.tensor_copy(
                            coll[ch // 2][r * 32:r * 32 + 1, :], pv[64:65, :]
                        )
                        o = opool.tile([64, CH], F32, tag="ost")
                        nc.vector.tensor_copy(o, pv[0:64, :])
                    ost[2 * ch + h] = o

            def emit_passB(chs):
                # pass B: group 3's contribution + combine + store (fp16
                # partials; the host sums the two partials of each pair)
                for e in range(NE):
                    ec = slice(e * P, (e + 1) * P)
                    for ch in chs:
                        psy = ps_acc.tile([P, CH], F32, tag="acc",
                                          name=f"pb{e}_{ch}")
                        nc.tensor.matmul(
                            psy, lhsT=wp_sb[:, 3, ec],
                            rhs=ot_sb[:, 3 * T + ch * CH:3 * T + (ch + 1) * CH],
                            start=True, stop=True,
                        )
                        ysb = ypool.tile([P, CH], F16, tag="ysb")
                        nc.vector.tensor_add(ysb, y_parts[(e, ch)], psy)
                        nc.sync.dma_start(
                            y[e * P:(e + 1) * P, ch * CH:(ch + 1) * CH], ysb
                        )

            groups = [list(range(num_devices))]
            ag_outs = {}

            def half_finalize(g, hf, coll, ost):
                """Normalize token-half hf of group g (batched reciprocal)
                and kick its AllGather."""
                nc.vector.reciprocal(coll[hf], coll[hf])
                for ch in (2 * hf, 2 * hf + 1):
                    for h in range(2):
                        r = 2 * (ch % 2) + h
                        rs = rpool.tile([1, CH], F32, tag="rs")
                        nc.vector.tensor_copy(
                            rs, coll[hf][r * 32:r * 32 + 1, :]
                        )
                        rb = rpool.tile([64, CH], F32, tag="rb")
                        nc.gpsimd.partition_broadcast(rb, rs)
                        cols = g * T + ch * CH
                        nc.vector.tensor_mul(
                            ot_sb[h * 64:(h + 1) * 64, cols:cols + CH],
                            ost[2 * ch + h], rb,
                        )
                if not use_a2a:
                    return
                ag_in = dpool.tile([P, TH], BF16, tag=f"ag_in{g}_{hf}")
                nc.sync.dma_start(
                    ag_in, ot_sb[:, g * T + hf * TH:g * T + (hf + 1) * TH]
                )
                nv = num_devices
                nc.gpsimd.collective_compute(
                    "AllGather",
                    mybir.AluOpType.bypass,
                    replica_groups=groups,
                    ins=[ag_in.opt()],
                    outs=[ag_outs[g][hf * nv:(hf + 1) * nv].opt()],
                )
                if hf == 1:
                    # both halves gathered: load the at_sb d-tiles this core
                    # needs: rows (own_half*nv + pair_base + j) of the
                    # [2*nv, P, TH] gather buffer, dynamic by rank.
                    for j in range(2):
                        nc.sync.dma_start(
                            at_sb[:, 4 * j + g, :],
                            ag_outs[g][bass.ds(row_base + j, 1)],
                        )

            if use_a2a:
                pid_sv = nc.sync.partition_id()
                own_half = pid_sv % 2
                # row own_half*nv + (pid - own_half) + j = pid + own_half*(nv-1) + j
                row_base = pid_sv + own_half * (num_devices - 1)
                for g in range(NG):
                    # [token-half x rank, P, TH]
                    ag_outs[g] = dpool.tile(
                        [2 * num_devices, P, TH], BF16,
                        tag=f"ag_out{g}", name=f"ago{g}"
                    )

            # ---- emission ----
            alloc_group(0)
            for _ in qkv_gen(0):
                pass

            for g in range(NG):
                if g + 1 < NG:
                    alloc_group(g + 1)
                    gen, n_ops = qkv_gen(g + 1), 3 * NCH * (ND + 1) + NTK * 2
                elif use_a2a:
                    gen, n_ops = proj_a_gen(), NE * 2 * 5
                else:
                    gen, n_ops = proj_a_gen(), NE * NCH * 4
                emitted = [0]

                def feed(i, gen=gen, n_ops=n_ops, emitted=emitted):
                    target = n_ops * (i + 1) // (NCH * NTK)
                    while emitted[0] < target:
                        next(gen, None)
                        emitted[0] += 1

                coll = [
                    collpool.tile([97, CH], F32, tag="coll0", name=f"coll0_{g}"),
                    collpool.tile([97, CH], F32, tag="coll1", name=f"coll1_{g}"),
                ]
                nc.gpsimd.memset(coll[0], 1.0)
                nc.gpsimd.memset(coll[1], 1.0)
                ost = {}
                for ch in range(NCH):
                    attn_chunk(g, ch, coll, ost, feed)
                    if not use_a2a and g == NG - 1 and ch == 2:
                        emit_passB((0, 1))
                    if ch == 1:
                        half_finalize(g, 0, coll, ost)
                    elif ch == 3:
                        half_finalize(g, 1, coll, ost)
                for _ in gen:
                    pass

            if use_a2a:
                # projection pass B: last group's d-tiles + combine + store
                for e in range(NE):
                    ec = slice(e * P, (e + 1) * P)
                    for chh in range(2):
                        psy = ps_acc.tile([P, CH], F32, tag="acc",
                                          name=f"pb{e}_{chh}")
                        dseq = (2, 6, 3, 7)
                        for k, dd in enumerate(dseq):
                            nc.tensor.matmul(
                                psy, lhsT=wp_sb[:, dd, ec],
                                rhs=at_sb[:, dd, chh * CH:(chh + 1) * CH],
                                start=(k == 0), stop=(k == len(dseq) - 1),
                            )
                        ysb = ypool.tile([P, CH], F32, tag="ysb")
                        nc.vector.tensor_add(ysb, y_parts[(e, chh)], psy)
                        nc.sync.dma_start(
                            y[e * P:(e + 1) * P, chh * CH:(chh + 1) * CH], ysb
                        )
            else:
                emit_passB((2, 3))

    nc.compile()
    return nc


def shard_inputs(x, w_qkv, w_proj, use_a2a=True):
    """Build the 8 per-core in_maps (host-side sharding + transposes)."""
    bf16 = ml_dtypes.bfloat16
    wp_t = np.ascontiguousarray(w_proj.T).astype(bf16)  # [d, e]
    in_maps = []
    for c in range(N_CORES):
        b, s = divmod(c, 2)
        xt = np.ascontiguousarray(x[b].T).astype(bf16)  # [D, T]
        heads = [8 * s + 2 * g for g in range(NG)]

        def wslice(base):
            cols = [
                w_qkv[base + h * DH: base + (h + 2) * DH, :] for h in heads
            ]
            return np.ascontiguousarray(np.concatenate(cols, axis=0).T).astype(bf16)

        m = {
            "xt": xt,
            "wq": wslice(0),
            "wk": wslice(D),
            "wv": wslice(2 * D),
        }
        if use_a2a:
            m["wp"] = wp_t
        else:
            rows = np.concatenate(
                [w_proj[:, (8 * s + 2 * g) * DH:(8 * s + 2 * g + 2) * DH].T
                 for g in range(NG)], axis=0
            )
            m["wp"] = np.ascontiguousarray(rows).astype(bf16)
        in_maps.append(m)
    return in_maps


def assemble_output(results, use_a2a=True):
    out = np.empty((4, T, D), dtype=np.float32)
    if use_a2a:
        for c in range(N_CORES):
            b, s = divmod(c, 2)
            out[b, s * TH:(s + 1) * TH, :] = results[c]["y"].T
    else:
        for b in range(4):
            acc = (results[2 * b]["y"].astype(np.float32)
                   + results[2 * b + 1]["y"].astype(np.float32))
            out[b] = acc.T
    return out


def run(x, w_qkv, w_proj, use_a2a=False, trace=False):
    key = ("k", use_a2a)
    if key not in _CACHE:
        _CACHE[key] = build_kernel(use_a2a=use_a2a)
    nc = _CACHE[key]
    in_maps = shard_inputs(x, w_qkv, w_proj, use_a2a=use_a2a)
    res = run_bass_kernel_spmd(
        nc, in_maps, core_ids=list(range(N_CORES)), trace=trace
    )
    return assemble_output(res.results, use_a2a=use_a2a), res


def kernel(x, w_qkv, w_proj):
    x = np.asarray(x, dtype=np.float32)
    w_qkv = np.asarray(w_qkv, dtype=np.float32)
    w_proj = np.asarray(w_proj, dtype=np.float32)
    out, _ = run(x, w_qkv, w_proj)
    return out
